# revision 18
# baseline (speedup 1.0000x reference)
"""Trainium2 Bass kernel for CvtLstm first-step (h=c=0) — full-IO contract.

Sharding: data-parallel, one batch sample per NeuronCore (N=8, 8 cores).

Per-core math (sample n):
  q  = conv3x3_same (x, Wq_eff)   with Wq_eff = w_qx . w_in  (host-folded 1x1)
  k  = conv3x3_valid(x, Wk_eff)
  vT = conv3x3_valid(x, Wv_eff) produced directly transposed [D, A]
  sT[d,q] = k.q per head — 4 heads concurrently via 32-row PE tiles (fp16)
  expT = exp(sT) on ACT, batched 3 ring slots per call (fp32 psum -> bf16)
  AV + Z: 4-way col-tiled matmuls per slot; head h lands at partitions
  32h..32h+32 (= its gate-channel range), Z replicated alongside
  a = araw * recip(Z), gates = 1x1 convs (bf16 a + fp16 x skip),
  c = sig(gi)*tanh(gg), hn = sig(go)*tanh(c), out = 1x1 conv + b_out.

Precision (validated vs fp64 reference, rel err ~4e-3): fp16 for x/q/k
(exp amplifies score error), bf16 for exp/v/gates (range needs bf16),
fp32 PSUM accumulation everywhere.

The last d-tile (4 rows, 1156 = 9*128+4) is packed: its 4 heads' scores
go to one ring slot at partition offsets 32h (diagonal PE tiles), so its
exp costs one q-chunk instead of four.
"""
import sys
import numpy as np
import ml_dtypes

sys.path.insert(0, '/opt/trn_rl_repo')

import concourse.bass as bass  # noqa: E402
import concourse.tile as tile  # noqa: E402
from concourse import bacc, mybir  # noqa: E402
from concourse.bass_utils import run_bass_kernel_spmd  # noqa: E402

F32 = mybir.dt.float32
F16 = mybir.dt.float16
BF16 = mybir.dt.bfloat16
AF = mybir.ActivationFunctionType

N, I, R, A, HEADS, O, H, W = 8, 128, 256, 256, 8, 256, 36, 36
HC = A // HEADS            # 32
Q = H * W                  # 1296
HK = H - 2                 # 34
D = HK * HK                # 1156
G = 3 * R                  # 768 gate channels kept (gi, gg, go)
HP = 38                    # padded width
XROWS = 39                 # padded rows (one slack row)
XPADF = XROWS * HP

QC_CHUNKS = [(0, 512), (512, 512), (1024, 272)]        # attention q chunks
DC_CHUNKS = [(0, 512), (512, 512), (1024, 132)]        # k-conv d chunks
QR_CHUNKS = [(0, 12), (12, 12), (24, 12)]              # q-conv row chunks

_CACHE = {}


def _build():
    nc = bacc.Bacc("TRN2", target_bir_lowering=False, debug=False)

    xc_d = nc.dram_tensor("xc", [128, 38 * HP], F16,
                          kind="ExternalInput").ap()
    wq_d = nc.dram_tensor("wq", [128, 9, 256], F16, kind="ExternalInput").ap()
    wk_d = nc.dram_tensor("wk", [128, 9, 256], F16, kind="ExternalInput").ap()
    wv_d = nc.dram_tensor("wv", [128, 9, 256], F16, kind="ExternalInput").ap()
    wga_d = nc.dram_tensor("wga", [2, 128, G], BF16,
                           kind="ExternalInput").ap()
    wgx_d = nc.dram_tensor("wgx", [128, G], F16, kind="ExternalInput").ap()
    wout_d = nc.dram_tensor("wout", [2, 128, 256], BF16,
                            kind="ExternalInput").ap()
    bg_d = nc.dram_tensor("bg", [128, 6], F32, kind="ExternalInput").ap()
    bo_d = nc.dram_tensor("bo", [128, 2], F32, kind="ExternalInput").ap()
    o_d = nc.dram_tensor("o", [256, Q], F32, kind="ExternalOutput").ap()

    with tile.TileContext(nc) as tc:
        with (
            tc.tile_pool(name="consts", bufs=1) as consts,
            tc.tile_pool(name="big", bufs=1) as big,
        ):
            # ---- input image + conv weights first (critical path) ----
            # x arrives pre-padded from the host ([38, 38] with zero border)
            # so no memset gates the DMA; row-chunked so the first conv
            # chunk can start before the whole image lands
            xpad = big.tile([128, XPADF], F16)
            XV = xpad.rearrange("p (r c) -> p r c", c=HP)  # [128, 39, 38]
            XCV = xc_d.rearrange("p (r c) -> p r c", c=HP)
            for r0, r1 in ((0, 15), (15, 27), (27, 38)):
                nc.sync.dma_start(XV[:, r0:r1, :], XCV[:, r0:r1, :])
            wq_t = consts.tile([128, 9, 256], F16)
            nc.sync.dma_start(wq_t[:], wq_d[:])
            wk_t = consts.tile([128, 9, 256], F16)
            nc.sync.dma_start(wk_t[:], wk_d[:])
            wv_t = consts.tile([128, 9, 256], F16)
            nc.sync.dma_start(wv_t[:], wv_d[:])

            ones_t = consts.tile([128, 32], BF16)
            nc.gpsimd.memset(ones_t[:], 1.0)
            # warm the ACT exp table while convs run
            edum = consts.tile([128, 2], F32)
            nc.scalar.activation(edum[:], ones_t[:, 0:2], AF.Exp)
            # warm the PE (HAM un-throttle needs ~3.4us of sustained matmul
            # activity) while the input DMAs are in flight
            wrm = consts.tile([128, 512], F16)
            nc.gpsimd.memset(wrm[:], 0.5)

            # gate/output weights (needed later; DMA'd behind the above)
            wga_t = consts.tile([128, 2, G], BF16)
            nc.sync.dma_start(wga_t[:, 0, :], wga_d[0])
            nc.sync.dma_start(wga_t[:, 1, :], wga_d[1])
            wgx_t = consts.tile([128, G], F16)
            nc.sync.dma_start(wgx_t[:], wgx_d[:])
            wout_t = consts.tile([128, 2, 256], BF16)
            nc.sync.dma_start(wout_t[:, 0, :], wout_d[0])
            nc.sync.dma_start(wout_t[:, 1, :], wout_d[1])
            bg_t = consts.tile([128, 6], F32)
            nc.sync.dma_start(bg_t[:], bg_d[:])
            bo_t = consts.tile([128, 2], F32)
            nc.sync.dma_start(bo_t[:], bo_d[:])

            # column-shifted compactions of x: for VALID-conv outputs the
            # operand index is linear in d (stride-34 rows), so any d-chunk
            # is a contiguous slice of xdx[dx]
            xdx = []
            for dx in range(3):
                t = big.tile([128, 37 * HK], F16, name=f"xdx{dx}")
                nc.vector.tensor_copy(
                    t.rearrange("p (r c) -> p r c", c=HK),
                    XV[:, 1:38, dx + 1:dx + 1 + HK])
                xdx.append(t)

            q_t = big.tile([128, 2, Q], F16)
            k_t = big.tile([128, 2, D], F16)
            vT_t = big.tile([128, 10, 8, HC], BF16)
            anorm = big.tile([128, 2, Q], BF16)

            # ---------------- q / k convolutions ----------------
            with tc.tile_pool(name="qkps", bufs=1, space="PSUM") as qkps:
                qp = qkps.tile([128, 2, 3, 512], F32, tag="qk")
                for i in range(9):
                    nc.tensor.matmul(qp[:, 0, 0, :], wrm[:, 0:128],
                                     wrm[:, :], start=True, stop=True)
                for at in range(2):
                    for s in range(9):
                        dy, dx = s // 3, s % 3
                        lhsT = wq_t[:, s, 128 * at:128 * at + 128]
                        for ci, (r0, nr) in enumerate(QR_CHUNKS):
                            rhs = XV[:, r0 + dy:r0 + dy + nr, dx:dx + W]
                            nc.tensor.matmul(
                                qp[:, at, ci, 0:nr * W], lhsT, rhs,
                                start=(s == 0), stop=(s == 8))
                    nc.vector.tensor_copy(
                        q_t[:, at, :].rearrange("p (ci w) -> p ci w", ci=3),
                        qp[:, at, :, 0:432])

                kp = qkps.tile([128, 2, 3, 512], F32, tag="qk")
                for at in range(2):
                    for s in range(9):
                        dy, dx = s // 3, s % 3
                        lhsT = wk_t[:, s, 128 * at:128 * at + 128]
                        for ci, (d0, dn) in enumerate(DC_CHUNKS):
                            rhs = xdx[dx][:, d0 + HK * dy:d0 + HK * dy + dn]
                            nc.tensor.matmul(
                                kp[:, at, ci, 0:dn], lhsT, rhs,
                                start=(s == 0), stop=(s == 8))
                    for ci, (d0, dn) in enumerate(DC_CHUNKS):
                        nc.vector.tensor_copy(
                            k_t[:, at, d0:d0 + dn], kp[:, at, ci, 0:dn])

            # ---------------- attention (+ overlapped v-conv) -------------
            # Per (at, q-chunk): scores batched 3 slots into fresh 3-bank
            # psum tiles (two alternate, giving two batches of ring slack);
            # slot = one (d-tile, head), 4 heads issued concurrently as
            # 32-row PE tiles; exp = one ACT call per batch. AV + Z are
            # col-tiled 4-way, head h accumulating at partitions 32h (its
            # gate-channel range).
            # Software pipeline at the chunk level: chunk i's AV/Z matmuls
            # are emitted interleaved with chunk i+1's scores, so the PE
            # always has dep-free work under the ACT-bound exp stream. The
            # v-conv tiles take that slot during the first chunk (its PSUM
            # banks are later reused for av/zb).
            with (
                tc.tile_pool(name="exps", bufs=16) as exps,
                tc.tile_pool(name="recs", bufs=2) as recs,
            ):
                ALL6 = [(at, qo, qn) for at in range(2)
                        for (qo, qn) in QC_CHUNKS]
                sgi = big.tile([128, 2, Q], BF16)
                tgg = big.tile([128, 2, Q], BF16)
                sgo = big.tile([128, 2, Q], BF16)
                c_t = big.tile([128, 2, Q], BF16)
                thc = big.tile([128, 2, Q], BF16)
                hn = big.tile([128, 2, Q], BF16)
                out_sb = big.tile([128, 2, Q], F32)
                MORDER = [(0, sgi, 0, AF.Sigmoid), (1, sgi, 1, AF.Sigmoid),
                          (4, sgo, 0, AF.Sigmoid), (5, sgo, 1, AF.Sigmoid),
                          (2, tgg, 0, AF.Tanh), (3, tgg, 1, AF.Tanh)]

                def gen_batches(at, qo, qn):
                    pend0 = 0
                    rng = None
                    for s in range(37):
                        if s == pend0:
                            rng = scps.tile([128, 3, 512], F32, tag="ring")
                        if s < 36:
                            dt, ha = s // 4, s % 4
                            nc.tensor.matmul(
                                rng[0:128, s - pend0, 0:qn],
                                k_t[32 * ha:32 * ha + 32, at,
                                    128 * dt:128 * dt + 128],
                                q_t[32 * ha:32 * ha + 32, at, qo:qo + qn],
                                start=True, stop=True,
                                tile_position=(32 * ha, 0))
                        else:
                            # ragged d-tile: 4 heads packed in one slot
                            for ha in range(4):
                                p0 = 32 * ha
                                nc.tensor.matmul(
                                    rng[p0:p0 + 4, s - pend0, 0:qn],
                                    k_t[p0:p0 + 32, at, 1152:1156],
                                    q_t[p0:p0 + 32, at, qo:qo + qn],
                                    start=True, stop=True,
                                    tile_position=(p0, p0))
                        if s - pend0 < 2 and s != 36:
                            continue
                        nb = s - pend0 + 1
                        e = exps.tile([128, 3, 512], BF16, tag="e")
                        nc.scalar.activation(
                            e[:, 0:nb, 0:qn], rng[:, 0:nb, 0:qn], AF.Exp)
                        yield (e, pend0, nb)
                        pend0 = s + 1

                def emit_avz(at, qn, av, zb, batch):
                    # all av matmuls first, then all zb: matmul starts are
                    # pc-monotone, so interleaving av/zb (same PE column
                    # group) would serialize the group
                    e, s0, nb = batch
                    for dst in (av, zb):
                        for j in range(nb):
                            s = s0 + j
                            if s < 36:
                                dt, ha = s // 4, s % 4
                                hg = 4 * at + ha
                                lhsT = (vT_t[:, dt, hg, :] if dst is av
                                        else ones_t[:, :])
                                nc.tensor.matmul(
                                    dst[32 * ha:32 * ha + 32, 0:qn],
                                    lhsT, e[:, j, 0:qn],
                                    start=(dt == 0), stop=False,
                                    tile_position=(0, 32 * ha))
                            else:
                                for ha in range(4):
                                    hg = 4 * at + ha
                                    p0 = 32 * ha
                                    lhsT = (vT_t[p0:p0 + 4, 9, hg, :]
                                            if dst is av
                                            else ones_t[p0:p0 + 4, :])
                                    nc.tensor.matmul(
                                        dst[p0:p0 + 32, 0:qn], lhsT,
                                        e[p0:p0 + 4, j, 0:qn],
                                        start=False, stop=True,
                                        tile_position=(p0, p0))

                def emit_vtile(j):
                    if j < 9:
                        d0 = 128 * j
                        vp = vps.tile([128, 256], F32, tag="v")
                        for s in range(9):
                            dy, dx = s // 3, s % 3
                            lhsT = xdx[dx][:,
                                           d0 + HK * dy:d0 + HK * dy + 128]
                            nc.tensor.matmul(
                                vp[:, :], lhsT, wv_t[:, s, :],
                                start=(s == 0), stop=(s == 8))
                        nc.vector.tensor_copy(
                            vT_t[:, j, :, :],
                            vp[:, :].rearrange("p (h c) -> p h c", c=HC))
                    else:
                        # ragged 4-row tile, one group per head slot
                        for ha in range(4):
                            vp = vps.tile([128, 256], F32, tag="v")
                            for s in range(9):
                                dy, dx = s // 3, s % 3
                                lhsT = xdx[dx][:,
                                               1152 + HK * dy:
                                               1156 + HK * dy]
                                nc.tensor.matmul(
                                    vp[32 * ha:32 * ha + 4, :], lhsT,
                                    wv_t[:, s, :],
                                    start=(s == 0), stop=(s == 8),
                                    tile_position=(0, 32 * ha))
                            for at_ in range(2):
                                p1 = 128 * at_ + 32 * ha
                                nc.vector.tensor_copy(
                                    vT_t[32 * ha:32 * ha + 4, 9,
                                         4 * at_ + ha, :],
                                    vp[32 * ha:32 * ha + 4, p1:p1 + 32])

                def finish_chunk(at, qo, qn, av, zb):
                    rec = recs.tile([128, 512], F32, tag="rec")
                    with nc.allow_low_precision(
                            reason="softmax normalizer reciprocal"):
                        nc.vector.reciprocal(rec[:, 0:qn], zb[:, 0:qn])
                    nc.vector.tensor_mul(
                        anorm[:, at, qo:qo + qn], av[:, 0:qn], rec[:, 0:qn])

                gpt = {}

                def emit_gate_group(ci, gi_):
                    r0, nr = QR_CHUNKS[ci]
                    qo, qnn = r0 * W, nr * W
                    if gi_ == 0:
                        gpt[ci] = (gps.tile([128, 3, 512], F32, tag="g3",
                                            name=f"gpA{ci}"),
                                   gps.tile([128, 3, 512], F32, tag="g3",
                                            name=f"gpB{ci}"))
                    m, dst, mm, fn = MORDER[gi_]
                    gp = gpt[ci][gi_ // 3]
                    gsl = gp[:, gi_ % 3, 0:qnn]
                    nc.tensor.matmul(
                        gsl, wga_t[:, 0, 128 * m:128 * m + 128],
                        anorm[:, 0, qo:qo + qnn], start=True, stop=False)
                    nc.tensor.matmul(
                        gsl, wga_t[:, 1, 128 * m:128 * m + 128],
                        anorm[:, 1, qo:qo + qnn], start=False, stop=False)
                    nc.tensor.matmul(
                        gsl, wgx_t[:, 128 * m:128 * m + 128],
                        XV[:, r0 + 1:r0 + 1 + nr, 1:37],
                        start=False, stop=True)
                    nc.scalar.activation(
                        dst[:, mm, qo:qo + qnn], gsl, fn,
                        bias=bg_t[:, m:m + 1])

                def emit_cell(ci):
                    r0, nr = QR_CHUNKS[ci]
                    qo, qnn = r0 * W, nr * W
                    csl = (slice(None), slice(None), slice(qo, qo + qnn))
                    nc.vector.tensor_mul(c_t[csl], sgi[csl], tgg[csl])
                    nc.scalar.activation(thc[csl], c_t[csl], AF.Tanh)
                    nc.vector.tensor_mul(hn[csl], sgo[csl], thc[csl])

                with tc.tile_pool(name="scps", bufs=2,
                                  space="PSUM") as scps:
                    with tc.tile_pool(name="vps", bufs=2,
                                      space="PSUM") as vps:
                        prev_b = []
                        for j, b in enumerate(gen_batches(*ALL6[0])):
                            prev_b.append(b)
                            if j <= 9:
                                emit_vtile(j)
                    prev = (ALL6[0], prev_b)

                    with tc.tile_pool(name="avzs", bufs=1,
                                      space="PSUM") as avzs:
                        for idx in range(1, 6):
                            (pat, pqo, pqn), pb = prev
                            av = avzs.tile([128, 512], F32, tag="av")
                            zb = avzs.tile([128, 512], F32, tag="zb")
                            cur = []
                            # lag the interleave 2 batches: av/zb alias
                            # the previous chunk's banks, so batch 0
                            # WAR-waits its rec/mul; the lag keeps that
                            # off the in-order PE queue until cleared
                            for j, b in enumerate(gen_batches(*ALL6[idx])):
                                cur.append(b)
                                if j >= 2 and j - 2 < len(pb):
                                    emit_avz(pat, pqn, av, zb, pb[j - 2])
                            for b in pb[max(0, len(cur) - 2):]:
                                emit_avz(pat, pqn, av, zb, b)
                            finish_chunk(pat, pqo, pqn, av, zb)
                            prev = (ALL6[idx], cur)

                # last chunk's AV/Z flush interleaves with the gate
                # matmuls for the already-normalized column chunks
                # (scores psum closed -> its banks host the gate psum)
                (lat, lqo, lqn), lb = prev
                with tc.tile_pool(name="avz2", bufs=1,
                                  space="PSUM") as avz2:
                    av = avz2.tile([128, 512], F32, tag="av")
                    zb = avz2.tile([128, 512], F32, tag="zb")
                    with tc.tile_pool(name="gps", bufs=2,
                                      space="PSUM") as gps:
                        k = 0
                        for j, b in enumerate(lb):
                            emit_avz(lat, lqn, av, zb, b)
                            if k < 12:
                                emit_gate_group(k // 6, k % 6)
                                k += 1
                                if k == 6:
                                    emit_cell(0)
                                elif k == 12:
                                    emit_cell(1)
                        finish_chunk(lat, lqo, lqn, av, zb)
                        for gi_ in range(6):
                            emit_gate_group(2, gi_)
                        emit_cell(2)

                with tc.tile_pool(name="ops", bufs=1, space="PSUM") as ops:
                    for ci, (r0, nr) in enumerate(QR_CHUNKS):
                        qo, qnn = r0 * W, nr * W
                        op = ops.tile([128, 2, 512], F32, tag="o")
                        for ot in range(2):
                            for rt in range(2):
                                nc.tensor.matmul(
                                    op[:, ot, 0:qnn],
                                    wout_t[:, rt, 128 * ot:128 * ot + 128],
                                    hn[:, rt, qo:qo + qnn],
                                    start=(rt == 0), stop=(rt == 1))
                            nc.vector.tensor_scalar_add(
                                out_sb[:, ot, qo:qo + qnn],
                                op[:, ot, 0:qnn], bo_t[:, ot:ot + 1])
                            nc.sync.dma_start(
                                o_d[128 * ot:128 * ot + 128, qo:qo + qnn],
                                out_sb[:, ot, qo:qo + qnn])

    nc.compile()
    return nc


def _prep(inputs):
    f8 = np.float64
    BF = ml_dtypes.bfloat16
    x = np.asarray(inputs['x'], np.float32)
    Wi = np.asarray(inputs['w_in'], f8)[:, :, 0, 0]           # [R, I]
    b_in = np.asarray(inputs['b_in'], f8)
    assert np.allclose(b_in, 0.0), "nonzero b_in unsupported by this build"

    def fold3(w):  # [A,R,3,3] x [R,I] -> [128 i, 9 s, 256 a] fp16
        we = np.einsum('arst,ri->aist', np.asarray(w, f8), Wi)
        return np.ascontiguousarray(
            we.transpose(1, 2, 3, 0).reshape(I, 9, A).astype(np.float16))

    wq = fold3(inputs['w_qx'])
    wk = fold3(inputs['w_kx'])
    wv = fold3(inputs['w_vx'])

    keep = np.r_[0:R, 2 * R:4 * R]                            # gi, gg, go
    Wga = np.asarray(inputs['w_ga'], f8)[:, :, 0, 0][keep]    # [G, A]
    wga = np.ascontiguousarray(Wga.T.reshape(2, 128, G).astype(BF))
    Wgx = np.asarray(inputs['w_gx'], f8)[:, :, 0, 0][keep]    # [G, R]
    Wgx_eff = Wgx @ Wi                                        # [G, I]
    wgx = np.ascontiguousarray(Wgx_eff.T.astype(np.float16))  # [128, G]
    b_eff = np.asarray(inputs['b_g'], f8)[keep] + Wgx @ b_in
    bg = np.ascontiguousarray(
        b_eff.reshape(6, 128).T.astype(np.float32))           # [128, 6]
    Wo = np.asarray(inputs['w_out'], f8)[:, :, 0, 0]          # [O, R]
    wout = np.ascontiguousarray(Wo.T.reshape(2, 128, 256).astype(BF))
    bo = np.ascontiguousarray(
        np.asarray(inputs['b_out'], f8).reshape(2, 128).T.astype(np.float32))

    shared = dict(wq=wq, wk=wk, wv=wv, wga=wga, wgx=wgx, wout=wout,
                  bg=bg, bo=bo)
    maps = []
    for n in range(N):
        xp = np.zeros((128, 38, 38), np.float16)
        xp[:, 1:37, 1:37] = x[n].reshape(128, 36, 36).astype(np.float16)
        maps.append(dict(shared, xc=xp.reshape(128, 38 * 38)))
    return maps


def get_nc():
    if 'nc' not in _CACHE:
        _CACHE['nc'] = _build()
    return _CACHE['nc']


def kernel(**inputs):
    nc = get_nc()
    in_maps = _prep(inputs)
    res = run_bass_kernel_spmd(nc, in_maps, core_ids=list(range(N)))
    out = np.stack([res.results[n]['o'].reshape(O, H, W) for n in range(N)])
    return out.astype(np.float32)


# revision 19
# speedup vs baseline: 1.1078x; 1.1078x over previous
"""Trainium2 Bass kernel for CvtLstm first-step (h=c=0) — full-IO contract.

Sharding: data-parallel, one batch sample per NeuronCore (N=8, 8 cores).

Per-core math (sample n):
  q  = conv3x3_same (x, Wq_eff)   with Wq_eff = w_qx . w_in  (host-folded 1x1)
  k  = conv3x3_valid(x, Wk_eff)
  vT = conv3x3_valid(x, Wv_eff) produced directly transposed [D, A]
  sT[d,q] = k.q per head — 4 heads concurrently via 32-row PE tiles (fp16)
  expT = exp(sT) on ACT, batched 3 ring slots per call (fp32 psum -> bf16)
  AV + Z: 4-way col-tiled matmuls per slot; head h lands at partitions
  32h..32h+32 (= its gate-channel range), Z replicated alongside
  a = araw * recip(Z), gates = 1x1 convs (bf16 a + fp16 x skip),
  c = sig(gi)*tanh(gg), hn = sig(go)*tanh(c), out = 1x1 conv + b_out.

Precision (validated vs fp64 reference, rel err ~4e-3): fp16 for x/q/k
(exp amplifies score error), bf16 for exp/v/gates (range needs bf16),
fp32 PSUM accumulation everywhere.

The last d-tile (4 rows, 1156 = 9*128+4) is packed: its 4 heads' scores
go to one ring slot at partition offsets 32h (diagonal PE tiles), so its
exp costs one q-chunk instead of four.
"""
import sys
import numpy as np
import ml_dtypes

sys.path.insert(0, '/opt/trn_rl_repo')

import concourse.bass as bass  # noqa: E402
import concourse.tile as tile  # noqa: E402
from concourse import bacc, mybir  # noqa: E402
from concourse.bass_utils import run_bass_kernel_spmd  # noqa: E402

F32 = mybir.dt.float32
F16 = mybir.dt.float16
BF16 = mybir.dt.bfloat16
AF = mybir.ActivationFunctionType

N, I, R, A, HEADS, O, H, W = 8, 128, 256, 256, 8, 256, 36, 36
HC = A // HEADS            # 32
Q = H * W                  # 1296
HK = H - 2                 # 34
D = HK * HK                # 1156
G = 3 * R                  # 768 gate channels kept (gi, gg, go)
HP = 38                    # padded width
XROWS = 39                 # padded rows (one slack row)
XPADF = XROWS * HP

QC_CHUNKS = [(0, 512), (512, 512), (1024, 272)]        # attention q chunks
DC_CHUNKS = [(0, 512), (512, 512), (1024, 132)]        # k-conv d chunks
QR_CHUNKS = [(0, 12), (12, 12), (24, 12)]              # q-conv row chunks

_CACHE = {}


def _build():
    nc = bacc.Bacc("TRN2", target_bir_lowering=False, debug=False)

    xc_d = nc.dram_tensor("xc", [128, 38 * HP], F16,
                          kind="ExternalInput").ap()
    wq_d = nc.dram_tensor("wq", [128, 9, 256], F16, kind="ExternalInput").ap()
    wk_d = nc.dram_tensor("wk", [128, 9, 256], F16, kind="ExternalInput").ap()
    wv_d = nc.dram_tensor("wv", [128, 9, 256], F16, kind="ExternalInput").ap()
    wga_d = nc.dram_tensor("wga", [2, 128, G], BF16,
                           kind="ExternalInput").ap()
    wgx_d = nc.dram_tensor("wgx", [128, G], F16, kind="ExternalInput").ap()
    wout_d = nc.dram_tensor("wout", [2, 128, 256], BF16,
                            kind="ExternalInput").ap()
    bg_d = nc.dram_tensor("bg", [128, 6], F32, kind="ExternalInput").ap()
    bo_d = nc.dram_tensor("bo", [128, 2], F32, kind="ExternalInput").ap()
    o_d = nc.dram_tensor("o", [256, Q], F32, kind="ExternalOutput").ap()

    with tile.TileContext(nc) as tc:
        with (
            tc.tile_pool(name="consts", bufs=1) as consts,
            tc.tile_pool(name="big", bufs=1) as big,
        ):
            # ---- input image + conv weights first (critical path) ----
            # x arrives pre-padded from the host ([38, 38] with zero border)
            # so no memset gates the DMA; row-chunked so the first conv
            # chunk can start before the whole image lands
            xpad = big.tile([128, XPADF], F16)
            XV = xpad.rearrange("p (r c) -> p r c", c=HP)  # [128, 39, 38]
            XCV = xc_d.rearrange("p (r c) -> p r c", c=HP)
            for r0, r1 in ((0, 15), (15, 27), (27, 38)):
                nc.sync.dma_start(XV[:, r0:r1, :], XCV[:, r0:r1, :])
            wq_t = consts.tile([128, 9, 256], F16)
            nc.sync.dma_start(wq_t[:], wq_d[:])
            wk_t = consts.tile([128, 9, 256], F16)
            nc.sync.dma_start(wk_t[:], wk_d[:])
            wv_t = consts.tile([128, 9, 256], F16)
            nc.sync.dma_start(wv_t[:], wv_d[:])

            # warm the PE (HAM un-throttle needs ~3.4us of sustained matmul
            # activity) while the input DMAs are in flight
            wrm = consts.tile([128, 512], F16)
            nc.gpsimd.memset(wrm[:], 0.5)
            ones_t = consts.tile([128, 32], BF16)
            nc.gpsimd.memset(ones_t[:], 1.0)
            # warm the ACT exp table while convs run
            edum = consts.tile([128, 2], F32)
            nc.scalar.activation(edum[:], ones_t[:, 0:2], AF.Exp)

            # gate/output weights (needed later; DMA'd behind the above)
            wga_t = consts.tile([128, 2, G], BF16)
            nc.sync.dma_start(wga_t[:, 0, :], wga_d[0])
            nc.sync.dma_start(wga_t[:, 1, :], wga_d[1])
            wgx_t = consts.tile([128, G], F16)
            nc.sync.dma_start(wgx_t[:], wgx_d[:])
            wout_t = consts.tile([128, 2, 256], BF16)
            nc.sync.dma_start(wout_t[:, 0, :], wout_d[0])
            nc.sync.dma_start(wout_t[:, 1, :], wout_d[1])
            bg_t = consts.tile([128, 6], F32)
            nc.sync.dma_start(bg_t[:], bg_d[:])
            bo_t = consts.tile([128, 2], F32)
            nc.sync.dma_start(bo_t[:], bo_d[:])

            # column-shifted compactions of x: for VALID-conv outputs the
            # operand index is linear in d (stride-34 rows), so any d-chunk
            # is a contiguous slice of xdx[dx]
            xdx = []
            for dx in range(3):
                t = big.tile([128, 37 * HK], F16, name=f"xdx{dx}")
                nc.vector.tensor_copy(
                    t.rearrange("p (r c) -> p r c", c=HK),
                    XV[:, 1:38, dx + 1:dx + 1 + HK])
                xdx.append(t)

            q_t = big.tile([128, 2, Q], F16)
            k_t = big.tile([128, 2, D], F16)
            vT_t = big.tile([128, 10, 8, HC], BF16)
            anorm = big.tile([128, 2, Q], BF16)

            # ---------------- q / k convolutions ----------------
            with tc.tile_pool(name="qkps", bufs=1, space="PSUM") as qkps:
                qp = qkps.tile([128, 2, 3, 512], F32, tag="qk")
                for i in range(12):
                    nc.tensor.matmul(qp[:, 0, 0, :], wrm[:, 0:128],
                                     wrm[:, :], start=True, stop=True)
                for at in range(2):
                    for s in range(9):
                        dy, dx = s // 3, s % 3
                        lhsT = wq_t[:, s, 128 * at:128 * at + 128]
                        for ci, (r0, nr) in enumerate(QR_CHUNKS):
                            rhs = XV[:, r0 + dy:r0 + dy + nr, dx:dx + W]
                            nc.tensor.matmul(
                                qp[:, at, ci, 0:nr * W], lhsT, rhs,
                                start=(s == 0), stop=(s == 8))
                    nc.vector.tensor_copy(
                        q_t[:, at, :].rearrange("p (ci w) -> p ci w", ci=3),
                        qp[:, at, :, 0:432])

                kp = qkps.tile([128, 2, 3, 512], F32, tag="qk")
                for at in range(2):
                    for s in range(9):
                        dy, dx = s // 3, s % 3
                        lhsT = wk_t[:, s, 128 * at:128 * at + 128]
                        for ci, (d0, dn) in enumerate(DC_CHUNKS):
                            rhs = xdx[dx][:, d0 + HK * dy:d0 + HK * dy + dn]
                            nc.tensor.matmul(
                                kp[:, at, ci, 0:dn], lhsT, rhs,
                                start=(s == 0), stop=(s == 8))
                    for ci, (d0, dn) in enumerate(DC_CHUNKS):
                        nc.vector.tensor_copy(
                            k_t[:, at, d0:d0 + dn], kp[:, at, ci, 0:dn])

            # ---------------- attention (+ overlapped v-conv) -------------
            # Per (at, q-chunk): scores batched 3 slots into fresh 3-bank
            # psum tiles (two alternate, giving two batches of ring slack);
            # slot = one (d-tile, head), 4 heads issued concurrently as
            # 32-row PE tiles; exp = one ACT call per batch. AV + Z are
            # col-tiled 4-way, head h accumulating at partitions 32h (its
            # gate-channel range).
            # Software pipeline at the chunk level: chunk i's AV/Z matmuls
            # are emitted interleaved with chunk i+1's scores, so the PE
            # always has dep-free work under the ACT-bound exp stream. The
            # v-conv tiles take that slot during the first chunk (its PSUM
            # banks are later reused for av/zb).
            with (
                tc.tile_pool(name="exps", bufs=16) as exps,
                tc.tile_pool(name="recs", bufs=2) as recs,
            ):
                ALL6 = [(at, qo, qn) for at in range(2)
                        for (qo, qn) in QC_CHUNKS]
                sgi = big.tile([128, 2, Q], BF16)
                tgg = big.tile([128, 2, Q], BF16)
                sgo = big.tile([128, 2, Q], BF16)
                c_t = big.tile([128, 2, Q], BF16)
                thc = big.tile([128, 2, Q], BF16)
                hn = big.tile([128, 2, Q], BF16)
                out_sb = big.tile([128, 2, Q], F32)
                MORDER = [(0, sgi, 0, AF.Sigmoid), (1, sgi, 1, AF.Sigmoid),
                          (4, sgo, 0, AF.Sigmoid), (5, sgo, 1, AF.Sigmoid),
                          (2, tgg, 0, AF.Tanh), (3, tgg, 1, AF.Tanh)]

                def gen_batches(at, qo, qn):
                    pend0 = 0
                    rng = None
                    for s in range(37):
                        if s == pend0:
                            rng = scps.tile([128, 3, 512], F32, tag="ring")
                        if s < 36:
                            dt, ha = s // 4, s % 4
                            nc.tensor.matmul(
                                rng[0:128, s - pend0, 0:qn],
                                k_t[32 * ha:32 * ha + 32, at,
                                    128 * dt:128 * dt + 128],
                                q_t[32 * ha:32 * ha + 32, at, qo:qo + qn],
                                start=True, stop=True,
                                tile_position=(32 * ha, 0))
                        else:
                            # ragged d-tile: 4 heads packed in one slot
                            for ha in range(4):
                                p0 = 32 * ha
                                nc.tensor.matmul(
                                    rng[p0:p0 + 4, s - pend0, 0:qn],
                                    k_t[p0:p0 + 32, at, 1152:1156],
                                    q_t[p0:p0 + 32, at, qo:qo + qn],
                                    start=True, stop=True,
                                    tile_position=(p0, p0))
                        if s - pend0 < 2 and s != 36:
                            continue
                        nb = s - pend0 + 1
                        e = exps.tile([128, 3, 512], BF16, tag="e")
                        nc.scalar.activation(
                            e[:, 0:nb, 0:qn], rng[:, 0:nb, 0:qn], AF.Exp)
                        yield (e, pend0, nb)
                        pend0 = s + 1

                def emit_avz(at, qn, av, zb, batch):
                    # all av matmuls first, then all zb: matmul starts are
                    # pc-monotone, so interleaving av/zb (same PE column
                    # group) would serialize the group
                    e, s0, nb = batch
                    for dst in (av, zb):
                        for j in range(nb):
                            s = s0 + j
                            if s < 36:
                                dt, ha = s // 4, s % 4
                                hg = 4 * at + ha
                                lhsT = (vT_t[:, dt, hg, :] if dst is av
                                        else ones_t[:, :])
                                nc.tensor.matmul(
                                    dst[32 * ha:32 * ha + 32, 0:qn],
                                    lhsT, e[:, j, 0:qn],
                                    start=(dt == 0), stop=False,
                                    tile_position=(0, 32 * ha))
                            else:
                                for ha in range(4):
                                    hg = 4 * at + ha
                                    p0 = 32 * ha
                                    lhsT = (vT_t[p0:p0 + 4, 9, hg, :]
                                            if dst is av
                                            else ones_t[p0:p0 + 4, :])
                                    nc.tensor.matmul(
                                        dst[p0:p0 + 32, 0:qn], lhsT,
                                        e[p0:p0 + 4, j, 0:qn],
                                        start=False, stop=True,
                                        tile_position=(p0, p0))

                def emit_vtile(j):
                    if j < 9:
                        d0 = 128 * j
                        vp = vps.tile([128, 256], F32, tag="v")
                        for s in range(9):
                            dy, dx = s // 3, s % 3
                            lhsT = xdx[dx][:,
                                           d0 + HK * dy:d0 + HK * dy + 128]
                            nc.tensor.matmul(
                                vp[:, :], lhsT, wv_t[:, s, :],
                                start=(s == 0), stop=(s == 8))
                        nc.vector.tensor_copy(
                            vT_t[:, j, :, :],
                            vp[:, :].rearrange("p (h c) -> p h c", c=HC))
                    else:
                        # ragged 4-row tile, one group per head slot
                        for ha in range(4):
                            vp = vps.tile([128, 256], F32, tag="v")
                            for s in range(9):
                                dy, dx = s // 3, s % 3
                                lhsT = xdx[dx][:,
                                               1152 + HK * dy:
                                               1156 + HK * dy]
                                nc.tensor.matmul(
                                    vp[32 * ha:32 * ha + 4, :], lhsT,
                                    wv_t[:, s, :],
                                    start=(s == 0), stop=(s == 8),
                                    tile_position=(0, 32 * ha))
                            for at_ in range(2):
                                p1 = 128 * at_ + 32 * ha
                                nc.vector.tensor_copy(
                                    vT_t[32 * ha:32 * ha + 4, 9,
                                         4 * at_ + ha, :],
                                    vp[32 * ha:32 * ha + 4, p1:p1 + 32])

                def finish_chunk(at, qo, qn, av, zb):
                    rec = recs.tile([128, 512], F32, tag="rec")
                    with nc.allow_low_precision(
                            reason="softmax normalizer reciprocal"):
                        nc.vector.reciprocal_approx_fast(rec[:, 0:qn],
                                                         zb[:, 0:qn])
                    nc.vector.tensor_mul(
                        anorm[:, at, qo:qo + qn], av[:, 0:qn], rec[:, 0:qn])

                gpt = {}

                def emit_gate_group(ci, gi_):
                    r0, nr = QR_CHUNKS[ci]
                    qo, qnn = r0 * W, nr * W
                    if gi_ == 0:
                        gpt[ci] = (gps.tile([128, 3, 512], F32, tag="g3",
                                            name=f"gpA{ci}"),
                                   gps.tile([128, 3, 512], F32, tag="g3",
                                            name=f"gpB{ci}"))
                    m, dst, mm, fn = MORDER[gi_]
                    gp = gpt[ci][gi_ // 3]
                    gsl = gp[:, gi_ % 3, 0:qnn]
                    nc.tensor.matmul(
                        gsl, wga_t[:, 0, 128 * m:128 * m + 128],
                        anorm[:, 0, qo:qo + qnn], start=True, stop=False)
                    nc.tensor.matmul(
                        gsl, wga_t[:, 1, 128 * m:128 * m + 128],
                        anorm[:, 1, qo:qo + qnn], start=False, stop=False)
                    nc.tensor.matmul(
                        gsl, wgx_t[:, 128 * m:128 * m + 128],
                        XV[:, r0 + 1:r0 + 1 + nr, 1:37],
                        start=False, stop=True)
                    nc.scalar.activation(
                        dst[:, mm, qo:qo + qnn], gsl, fn,
                        bias=bg_t[:, m:m + 1])

                def emit_cell(ci):
                    r0, nr = QR_CHUNKS[ci]
                    qo, qnn = r0 * W, nr * W
                    csl = (slice(None), slice(None), slice(qo, qo + qnn))
                    nc.vector.tensor_mul(c_t[csl], sgi[csl], tgg[csl])
                    nc.scalar.activation(thc[csl], c_t[csl], AF.Tanh)
                    nc.vector.tensor_mul(hn[csl], sgo[csl], thc[csl])

                with tc.tile_pool(name="scps", bufs=2,
                                  space="PSUM") as scps:
                    with tc.tile_pool(name="vps", bufs=2,
                                      space="PSUM") as vps:
                        prev_b = []
                        for j, b in enumerate(gen_batches(*ALL6[0])):
                            prev_b.append(b)
                            if j <= 9:
                                emit_vtile(j)
                    prev = (ALL6[0], prev_b)

                    with tc.tile_pool(name="avzs", bufs=1,
                                      space="PSUM") as avzs:
                        for idx in range(1, 6):
                            (pat, pqo, pqn), pb = prev
                            av = avzs.tile([128, 512], F32, tag="av")
                            zb = avzs.tile([128, 512], F32, tag="zb")
                            cur = []
                            # lag the interleave 2 batches: av/zb alias
                            # the previous chunk's banks, so batch 0
                            # WAR-waits its rec/mul; the lag keeps that
                            # off the in-order PE queue until cleared
                            for j, b in enumerate(gen_batches(*ALL6[idx])):
                                cur.append(b)
                                if j >= 2 and j - 2 < len(pb):
                                    emit_avz(pat, pqn, av, zb, pb[j - 2])
                            for b in pb[max(0, len(cur) - 2):]:
                                emit_avz(pat, pqn, av, zb, b)
                            finish_chunk(pat, pqo, pqn, av, zb)
                            prev = (ALL6[idx], cur)

                # last chunk's AV/Z flush interleaves with the gate
                # matmuls for the already-normalized column chunks
                # (scores psum closed -> its banks host the gate psum)
                (lat, lqo, lqn), lb = prev
                with tc.tile_pool(name="avz2", bufs=1,
                                  space="PSUM") as avz2:
                    av = avz2.tile([128, 512], F32, tag="av")
                    zb = avz2.tile([128, 512], F32, tag="zb")
                    with tc.tile_pool(name="gps", bufs=2,
                                      space="PSUM") as gps:
                        k = 0
                        for j, b in enumerate(lb):
                            emit_avz(lat, lqn, av, zb, b)
                            if k < 12:
                                emit_gate_group(k // 6, k % 6)
                                k += 1
                                if k == 6:
                                    emit_cell(0)
                                elif k == 12:
                                    emit_cell(1)
                        finish_chunk(lat, lqo, lqn, av, zb)
                        for gi_ in range(6):
                            emit_gate_group(2, gi_)
                        emit_cell(2)

                with tc.tile_pool(name="ops", bufs=1, space="PSUM") as ops:
                    for ci, (r0, nr) in enumerate(QR_CHUNKS):
                        qo, qnn = r0 * W, nr * W
                        op = ops.tile([128, 2, 512], F32, tag="o")
                        for ot in range(2):
                            for rt in range(2):
                                nc.tensor.matmul(
                                    op[:, ot, 0:qnn],
                                    wout_t[:, rt, 128 * ot:128 * ot + 128],
                                    hn[:, rt, qo:qo + qnn],
                                    start=(rt == 0), stop=(rt == 1))
                            nc.vector.tensor_scalar_add(
                                out_sb[:, ot, qo:qo + qnn],
                                op[:, ot, 0:qnn], bo_t[:, ot:ot + 1])
                            nc.sync.dma_start(
                                o_d[128 * ot:128 * ot + 128, qo:qo + qnn],
                                out_sb[:, ot, qo:qo + qnn])

    nc.compile()
    return nc


def _prep(inputs):
    f8 = np.float64
    BF = ml_dtypes.bfloat16
    x = np.asarray(inputs['x'], np.float32)
    Wi = np.asarray(inputs['w_in'], f8)[:, :, 0, 0]           # [R, I]
    b_in = np.asarray(inputs['b_in'], f8)
    assert np.allclose(b_in, 0.0), "nonzero b_in unsupported by this build"

    def fold3(w):  # [A,R,3,3] x [R,I] -> [128 i, 9 s, 256 a] fp16
        we = np.einsum('arst,ri->aist', np.asarray(w, f8), Wi)
        return np.ascontiguousarray(
            we.transpose(1, 2, 3, 0).reshape(I, 9, A).astype(np.float16))

    wq = fold3(inputs['w_qx'])
    wk = fold3(inputs['w_kx'])
    wv = fold3(inputs['w_vx'])

    keep = np.r_[0:R, 2 * R:4 * R]                            # gi, gg, go
    Wga = np.asarray(inputs['w_ga'], f8)[:, :, 0, 0][keep]    # [G, A]
    wga = np.ascontiguousarray(Wga.T.reshape(2, 128, G).astype(BF))
    Wgx = np.asarray(inputs['w_gx'], f8)[:, :, 0, 0][keep]    # [G, R]
    Wgx_eff = Wgx @ Wi                                        # [G, I]
    wgx = np.ascontiguousarray(Wgx_eff.T.astype(np.float16))  # [128, G]
    b_eff = np.asarray(inputs['b_g'], f8)[keep] + Wgx @ b_in
    bg = np.ascontiguousarray(
        b_eff.reshape(6, 128).T.astype(np.float32))           # [128, 6]
    Wo = np.asarray(inputs['w_out'], f8)[:, :, 0, 0]          # [O, R]
    wout = np.ascontiguousarray(Wo.T.reshape(2, 128, 256).astype(BF))
    bo = np.ascontiguousarray(
        np.asarray(inputs['b_out'], f8).reshape(2, 128).T.astype(np.float32))

    shared = dict(wq=wq, wk=wk, wv=wv, wga=wga, wgx=wgx, wout=wout,
                  bg=bg, bo=bo)
    maps = []
    for n in range(N):
        xp = np.zeros((128, 38, 38), np.float16)
        xp[:, 1:37, 1:37] = x[n].reshape(128, 36, 36).astype(np.float16)
        maps.append(dict(shared, xc=xp.reshape(128, 38 * 38)))
    return maps


def get_nc():
    if 'nc' not in _CACHE:
        _CACHE['nc'] = _build()
    return _CACHE['nc']


def kernel(**inputs):
    nc = get_nc()
    in_maps = _prep(inputs)
    res = run_bass_kernel_spmd(nc, in_maps, core_ids=list(range(N)))
    out = np.stack([res.results[n]['o'].reshape(O, H, W) for n in range(N)])
    return out.astype(np.float32)
